# revision 1
# baseline (speedup 1.0000x reference)
"""Chamfer 1D loss on 8 TRN2 NeuronCores — dual-tile squared-distance kernel.

Sharding: core c owns x[2048c:2048(c+1)] and y[2048c:2048(c+1)] as "row"
blocks; each direction's min is computed against the FULL other array
(replicated to every core), so no inter-core collective is needed — each
core emits per-row minima of SQUARED distances and the host takes
sqrt + sums (min_j |d| = sqrt(min_j d^2), exact up to fp rounding).

Per core, per direction: 2048 rows live one-per-partition-lane as 16
tiles of [128,1]; the full opposing array [16384] is partition-broadcast
into SBUF. One custom "dual-tile" DVE instruction per PAIR of row tiles
streams a column chunk once and computes BOTH tiles' running minima:

  s0: d_t = y - x_t     s3: d_u = y - x_u
  s1: d_t^2             s4: d_u^2
  s2: acc_t = min(..)   s5: acc_u = min(..)

i.e. 2 row-column pairs per element read per cycle — 2x the throughput
of the 1-elem/cycle fused op (the DVE's 2-read-port ceiling, reached
here with one port in plain REGULAR mode). The two accumulators live in
stage-2/stage-5 CURR_ALU_OUT flops, seeded from imm2 by a 1-count seed
uop and drained after SRC_TENSOR_DONE by two 1-count writer uops (the
stock FIND_INDEX_8 post-stream pattern).

Output per core: mins0 [128,16,2] (x-rows, 2 column chunks) and
mins1 [128,16] (y-rows, 1 chunk) of squared minima.
"""

import numpy as np

import concourse.bacc as bacc
import concourse.mybir as mybir
import concourse.tile as tile
import concourse.bass_utils as bass_utils

import concourse.bass_isa as bass_isa
import concourse.dve_ops as dve_ops
from concourse.dve_ops import get_dve_sub_opcode
from concourse.dve_spec import Spec, Src0, C0, C1, minn, sq
from concourse.dve_uop import (
    AluInp,
    AluOp,
    DelayInp,
    DveOpSpec,
    InpSel,
    OutPath,
    OutSel,
    Trigger,
    UopConfig,
)

F32 = mybir.dt.float32
P = 128          # partitions
NF = 16384       # full length of each input
NB = NF // 8     # row block per core (2048)
T = NB // P      # row tiles per block (16)
NPAIR = T // 2   # dual-tile instructions per (direction, chunk)
# Direction 0 streams the opposing array in two chunks so the first
# instruction starts as soon as ~2048 columns have broadcast; direction
# 1 uses one chunk whose broadcast fully overlaps direction-0 compute.
CHUNKS0 = [2048, 14336]
ALPHA = 0.5
BIG = 3.0e38

OP_NAME = "CHAMFER_SQD2_ANT"
_D = AluInp


def _dual_uops() -> list[UopConfig]:
    """[seed, steady, spacer, drain_t, drain_u].

    Lane map: D0=Src0 (column value), D1=CONST_0 (x_t), D2=CONST_1 (x_u),
    D3=CONST_2 (imm2 accumulator seed).
    """

    def route(u: UopConfig) -> UopConfig:
        u.enable_input(InpSel.SRC_0, 1)
        u.enable_input(InpSel.CONST_0, 2)
        u.enable_input(InpSel.CONST_1, 3)
        u.enable_input(InpSel.CONST_2, 4)
        return u

    # seed: CURR[s2] <- imm2, CURR[s5] <- imm2 (one bubble element)
    u0 = route(UopConfig())
    dp = u0.datapath_config
    for s in (0, 1):
        dp[s].pass_through_delay(3)
    dp[2].enable_alu(AluOp.BYPASS, _D.PREV_DELAY_3).pass_through_delay(3)
    for s in (3, 4):
        dp[s].pass_through_delay(3)
    dp[5].enable_alu(AluOp.BYPASS, _D.PREV_DELAY_3)
    u0.repeat_count = 1
    u0.trigger = (Trigger.COUNT, Trigger.NONE, Trigger.NONE)
    u0.next_uop = (1, 0, 0)

    # steady: both chains, one element consumed per cycle
    u1 = route(UopConfig())
    dp = u1.datapath_config
    dp[0].enable_alu(AluOp.SUBTRACT, _D.PREV_DELAY_0, _D.PREV_DELAY_1)
    dp[0].pass_through_delay(0, 2)
    dp[1].enable_alu(AluOp.MULTIPLY, _D.PREV_ALU_OUT, _D.PREV_ALU_OUT)
    dp[1].pass_through_delay(0, 2)
    dp[2].enable_alu(AluOp.MIN, _D.CURR_ALU_OUT, _D.PREV_ALU_OUT)
    dp[2].pass_through_delay(0, 2)
    dp[3].enable_alu(AluOp.SUBTRACT, _D.PREV_DELAY_0, _D.PREV_DELAY_2)
    dp[4].enable_alu(AluOp.MULTIPLY, _D.PREV_ALU_OUT, _D.PREV_ALU_OUT)
    dp[5].enable_alu(AluOp.MIN, _D.CURR_ALU_OUT, _D.PREV_ALU_OUT)
    u1.require_inp0 = 1
    u1.trigger = (Trigger.SRC_TENSOR_DONE, Trigger.NONE, Trigger.NONE)
    u1.next_uop = (2, 0, 0)

    # spacer: pure bubble (touches no flops) so the last real element
    # clears the accumulator stages before the drain elements read them
    u2 = route(UopConfig())
    u2.repeat_count = 1
    u2.trigger = (Trigger.COUNT, Trigger.NONE, Trigger.NONE)
    u2.next_uop = (3, 0, 0)

    # drain_t: emit CURR[s2] via delay lane 0 (s5's flop holds acc_u and
    # must not be written, so the value bypasses the ALU chain)
    u3 = route(UopConfig())
    dp = u3.datapath_config
    dp[2].enable_alu(AluOp.BYPASS, _D.CURR_ALU_OUT)
    dp[3].enable_delay_from_src(DelayInp.PREV_ALU_OUT, 0)
    for s in (4, 5, 6, 7):
        dp[s].pass_through_delay(0)
    u3.enable_output(OutSel.DELAY_0, OutPath.WR0_LO)
    u3.repeat_count = 1
    u3.trigger = (Trigger.COUNT, Trigger.NONE, Trigger.NONE)
    u3.next_uop = (4, 0, 0)

    # drain_u: emit CURR[s5] via the ALU chain
    u4 = route(UopConfig())
    dp = u4.datapath_config
    dp[5].enable_alu(AluOp.BYPASS, _D.CURR_ALU_OUT)
    dp[6].pass_through_alu()
    dp[7].pass_through_alu()
    u4.enable_output(OutSel.ALU_OUT, OutPath.WR0_LO)
    u4.repeat_count = 1
    u4.trigger = (Trigger.COUNT, Trigger.NONE, Trigger.NONE)
    u4.next_uop = (0, 0, 0)

    return [u0, u1, u2, u3, u4]


class _DualOp:
    """Duck-typed dve_ops.DveOp with a hand-written uop chain."""

    def __init__(self, name: str, spec: Spec):
        self.name = name
        self.spec = spec
        self.subdim = False
        self._cache: dict[str, DveOpSpec] = {}

    def compile(self, ver: str) -> DveOpSpec:
        if ver in self._cache:
            return self._cache[ver]
        assert ver == "v3", "kernel targets TRN2"
        s = DveOpSpec(
            name=self.name,
            opcode=get_dve_sub_opcode(self.name),
            uops=_dual_uops(),
            rd1_en=False,
        )
        self._cache[ver] = s
        return s


def _register() -> _DualOp:
    if OP_NAME in dve_ops._SUB_OPCODE_FOR_NAME:
        for op in dve_ops.OPS:
            if op.name == OP_NAME:
                return op
        raise RuntimeError("row allocated but op missing")
    # registry-compat spec (sims only; HW semantics come from _dual_uops)
    spec = Spec(body=sq(Src0 - C0), accum=minn, accum_init=C1)
    row = dve_ops._CUSTOM_DVE_ROW_BASE + len(dve_ops.OPS)
    assert row < 0x20
    dve_ops._SUB_OPCODE_FOR_NAME[OP_NAME] = row
    op = _DualOp(OP_NAME, spec)
    dve_ops.OPS.append(op)
    dve_ops.CUSTOM_DVE_SPECS[OP_NAME] = spec
    return op


SQD2 = _register()


def _emit(vec, *, out, in0, s0, s1):
    op = SQD2
    bassm = vec.bass
    if op.name not in bassm.m.ant_custom_dve_ops:
        bassm.m.ant_custom_dve_ops = sorted({*bassm.m.ant_custom_dve_ops, op.name})
    op.compile("v3")
    shape = bass_isa.CustomDveShape.TTSS
    opc = bassm.isa.Opcode[
        f"NEURON_ISA_TPB_OPCODE_CUSTOM_DVE_ANT_{shape.slot()}"
    ].value
    ins_l = [
        vec.lower_ap(in0, for_isa=True, opt=True),
        vec.lower_ap(s0, for_isa=True),
        vec.lower_ap(s1, for_isa=True),
    ]
    outs_l = [vec.lower_ap(out, for_isa=True)]
    return vec.add_instruction(
        bass_isa.InstCustomDveAnt(
            name=bassm.get_next_instruction_name(),
            op_name=op.name,
            rd1_en=False,
            subdim=0,
            imm2=BIG,
            shape=shape,
            row=get_dve_sub_opcode(op.name),
            isa_opcode=opc,
            ins=ins_l,
            outs=outs_l,
            perf_max=0,
        )
    )


_NC_CACHE = None


def _build():
    global _NC_CACHE
    if _NC_CACHE is not None:
        return _NC_CACHE
    nc = bacc.Bacc("TRN2", target_bir_lowering=False, debug=False, num_devices=8)
    x_blk = nc.dram_tensor("x_blk", [NB], F32, kind="ExternalInput")
    y_blk = nc.dram_tensor("y_blk", [NB], F32, kind="ExternalInput")
    x_full = nc.dram_tensor("x_full", [NF], F32, kind="ExternalInput")
    y_full = nc.dram_tensor("y_full", [NF], F32, kind="ExternalInput")
    mins0 = nc.dram_tensor("mins0", [P, T, 2], F32, kind="ExternalOutput")
    mins1 = nc.dram_tensor("mins1", [P, T], F32, kind="ExternalOutput")

    with tile.TileContext(nc) as tc:
        with (
            tc.tile_pool(name="bcast0", bufs=1) as bc_pool0,
            tc.tile_pool(name="bcast1", bufs=1) as bc_pool1,
            tc.tile_pool(name="small", bufs=1) as small,
        ):
            # All input DMAs ride the SP ring IN ORDER. Split queues lose:
            # concurrent queues round-robin per descriptor, so a small early
            # chunk gets starved behind the big broadcasts. On one queue
            # bacc merges ADJACENT compatible transfers, which would make
            # the first instruction wait for the FULL y broadcast — so the
            # rows load for direction 0 sits BETWEEN the two y chunks to
            # break the merge; the queue then serves chunk 0 (~1MB) first.
            cols0 = []
            c0a = bc_pool0.tile([P, CHUNKS0[0]], F32, tag="c0_0")
            nc.sync.dma_start(
                c0a[:],
                y_full.ap()[0 : CHUNKS0[0]].unsqueeze(0).partition_broadcast(P),
            )
            cols0.append(c0a)
            rows0 = small.tile([P, T], F32, tag="rows0")
            nc.sync.dma_start(rows0[:], x_blk.ap().rearrange("(p t) -> p t", p=P))
            c0b = bc_pool0.tile([P, CHUNKS0[1]], F32, tag="c0_1")
            nc.sync.dma_start(
                c0b[:],
                y_full.ap()[CHUNKS0[0] :].unsqueeze(0).partition_broadcast(P),
            )
            cols0.append(c0b)
            rows1 = small.tile([P, T], F32, tag="rows1")
            nc.sync.dma_start(rows1[:], y_blk.ap().rearrange("(p t) -> p t", p=P))
            cols1 = bc_pool1.tile([P, NF], F32, tag="c1")
            nc.sync.dma_start(
                cols1[:], x_full.ap().unsqueeze(0).partition_broadcast(P)
            )

            minw0 = small.tile([P, T, 2], F32, tag="minw0")
            minw1 = small.tile([P, T], F32, tag="minw1")

            for ch in range(len(CHUNKS0)):
                for p in range(NPAIR):
                    _emit(
                        nc.vector,
                        out=minw0[:, 2 * p : 2 * p + 2, ch],
                        in0=cols0[ch][:],
                        s0=rows0[:, 2 * p : 2 * p + 1],
                        s1=rows0[:, 2 * p + 1 : 2 * p + 2],
                    )
            # dir-0 results drain on the ACT ring while dir-1 still computes
            nc.scalar.dma_start(mins0.ap(), minw0[:])
            for p in range(NPAIR):
                _emit(
                    nc.vector,
                    out=minw1[:, 2 * p : 2 * p + 2],
                    in0=cols1[:],
                    s0=rows1[:, 2 * p : 2 * p + 1],
                    s1=rows1[:, 2 * p + 1 : 2 * p + 2],
                )
            nc.sync.dma_start(mins1.ap(), minw1[:])
    nc.compile()
    _NC_CACHE = nc
    return nc


def kernel(**inputs: np.ndarray) -> np.ndarray:
    x = np.ascontiguousarray(inputs["inputs"], dtype=np.float32).reshape(-1)
    y = np.ascontiguousarray(inputs["targets"], dtype=np.float32).reshape(-1)
    assert x.shape == (NF,) and y.shape == (NF,)

    nc = _build()
    in_maps = [
        {
            "x_blk": x[c * NB : (c + 1) * NB],
            "y_blk": y[c * NB : (c + 1) * NB],
            "x_full": x,
            "y_full": y,
        }
        for c in range(8)
    ]
    res = bass_utils.run_bass_kernel_spmd(nc, in_maps, core_ids=list(range(8)))

    cd_xy = 0.0
    cd_yx = 0.0
    for c in range(8):
        m0 = res.results[c]["mins0"]  # [P, T, 2] squared minima per chunk
        m1 = res.results[c]["mins1"]  # [P, T]
        cd_xy += np.sqrt(m0.min(axis=2)).sum(dtype=np.float64)
        cd_yx += np.sqrt(m1).sum(dtype=np.float64)
    val = ALPHA * cd_xy / NF + (1.0 - ALPHA) * cd_yx / NF
    return np.float32(val)

